# revision 1
# baseline (speedup 1.0000x reference)
"""Distorted-SSIM loss kernel for Trainium2 (8 NeuronCores, data parallel).

Decomposition (per [512,512] plane; x=img1 plane, y=img2 plane):
    w1 = x+y -> blur = mu1+mu2 (S);  w2 = x-y -> blur = mu1-mu2 (D)
    up = x^2+y^2 -> blur = U;        vp = x*y -> blur = V
    sa = S^2/2, sb = D^2/2;  T' = sa-sb = 2 mu1 mu2;  q = sa+sb = mu1^2+mu2^2
    num = (T' + C1) * (2V - T' + C2);  den = (q + C1) * (U - q + C2)
    loss = mean(num/den) over pixels, windows (5,11),(11,5),(11,11), planes.

Convs are banded matmuls on TensorE over 5 overlapping 128-row windows
producing 118-row output chunks (no fringe matmuls; zero padding == band
truncation).  Stage 1 uses the image tile as the matmul stationary so the
column-conv result lands transposed; stage 2 then does the row conv with
the banded block stationary.  Everything on-chip is float16 (11-bit
mantissa needed: bf16's 8 bits bias the tiny C2-driven numerator), PSUM
accumulates fp32, division in fp32.  Gaussian taps are fp16-rounded then
ULP-nudged so they sum to exactly 1.0 (a tap-sum off by d biases E[sigma12]
by -0.25*d, comparable to C2).

Each core handles 4 images (12 planes) and returns per-w-chunk partial
sums [128, 5]; the host reduces.
"""

import sys
import numpy as np

for _p in ("/opt/trn_rl_repo",):
    if _p not in sys.path:
        sys.path.insert(0, _p)

SIGMA = 1.5
C1 = 0.01**2
C2 = 0.03**2

STARTS = [0, 113, 231, 349, 467]
NCH = 5
KSZ = [min(128, 512 - s) for s in STARTS]
MSZ = [118, 118, 118, 118, 40]
N_PLANES = 12


def _gaussian(n, sigma=SIGMA):
    x = np.arange(n, dtype=np.float64)
    g = np.exp(-((x - n // 2) ** 2) / (2.0 * sigma**2))
    return (g / g.sum()).astype(np.float32)


def _norm_fp16_taps(g):
    """fp16 taps ULP-nudged so the fp64 sum is exactly 1.0."""
    t = g.astype(np.float16)
    for _ in range(500):
        td = t.astype(np.float64)
        err = td.sum() - 1.0
        if abs(err) < 2e-8:
            break
        bits = t.view(np.uint16).astype(np.int32) + (1 if err < 0 else -1)
        stepped = bits.astype(np.uint16).view(np.float16)
        delta = stepped.astype(np.float64) - td
        ad = np.abs(delta)
        ok = ad <= abs(err) * 1.000001
        i = int(np.argmax(np.where(ok, ad, -1.0))) if ok.any() else int(np.argmin(ad))
        t[i] = stepped[i]
    return t


def _wblocks(k):
    """Banded conv blocks [128, 5, 118]: W[kk, c, m] = g[in - out + pad]."""
    g = _norm_fp16_taps(_gaussian(k)).astype(np.float32)
    p = k // 2
    W = np.zeros((128, NCH, 118), np.float32)
    kk = np.arange(128)
    for c, s in enumerate(STARTS):
        m = np.arange(MSZ[c])
        j = (s + kk[:, None]) - (118 * c + m[None, :]) + p
        valid = (j >= 0) & (j < k) & (kk[:, None] < KSZ[c])
        W[:, c, : MSZ[c]][valid] = g[np.clip(j, 0, k - 1)][valid]
    return W.astype(np.float16)


def _overlap_planes(pl):
    """[12, 512, 512] fp32 -> [12, 128, 5*512] fp16 overlapped h-window tiles."""
    t = np.zeros((N_PLANES, NCH, 128, 512), np.float32)
    for c, s in enumerate(STARTS):
        t[:, c, : KSZ[c], :] = pl[:, s : s + KSZ[c], :]
    return np.ascontiguousarray(
        t.transpose(0, 2, 1, 3).reshape(N_PLANES, 128, NCH * 512)
    ).astype(np.float16)


_PROGRAM = {}
FREE = NCH * 512  # 2560


def _build_program(reps=1, noop=False, stage=5):
    import concourse.bass as bass
    import concourse.mybir as mybir
    from concourse import bacc, tile

    f32 = mybir.dt.float32
    f16 = mybir.dt.float16
    Alu = mybir.AluOpType
    Act = mybir.ActivationFunctionType

    nc = bacc.Bacc(None, target_bir_lowering=False)
    xy_d = nc.dram_tensor("xyov", [N_PLANES, 128, 2 * FREE], f16, kind="ExternalInput")
    wb_d = nc.dram_tensor("wb", [128, 2, NCH, 118], f16, kind="ExternalInput")
    out_d = nc.dram_tensor("out", [128, 512], f16, kind="ExternalOutput")

    SQH = float(np.sqrt(0.5))

    with tile.TileContext(nc) as tc:
        with (
            tc.tile_pool(name="const", bufs=1) as cpool,
            tc.tile_pool(name="xy", bufs=2) as xypool,
            tc.tile_pool(name="maps", bufs=3) as mpool,
            tc.tile_pool(name="tmp", bufs=2) as tpool,
            tc.tile_pool(name="cmap", bufs=1) as cmpool,
            tc.tile_pool(name="win", bufs=2) as wpool,
            tc.tile_pool(name="ps1", bufs=2, space="PSUM") as ps1pool,
            tc.tile_pool(name="ps2", bufs=4, space="PSUM") as ps2pool,
        ):
            wb = cpool.tile([128, 2, NCH, 118], f16, tag="wb")
            nc.sync.dma_start(wb[:], wb_d[:])
            w5 = wb[:, 0]
            w11 = wb[:, 1]
            acc = cpool.tile([128, 512], f16, tag="acc")
            nc.vector.memset(acc[:], 0.0)

            # dummy matmul: absorb the wb DMA wait on PE once, so later
            # matmuls carry a single wait (their lhsT/rhs producer)
            dummy = ps2pool.tile([128, 512], f32, tag="ps2")
            nc.tensor.matmul(
                dummy[0:118, 0:118], wb[0:128, 0, 0, 0:118], wb[0:128, 0, 0, 0:118],
                start=True, stop=True,
            )


            plane_seq = [] if noop else [pp for _ in range(reps) for pp in range(N_PLANES)]
            for p in plane_seq:
                xy = xypool.tile([128, 2 * FREE], f16, tag="xy")
                nc.sync.dma_start(xy[:], xy_d[p])
                x = xy[:, 0:FREE]
                y = xy[:, FREE : 2 * FREE]

                w1 = mpool.tile([128, FREE], f16, tag="w1")
                w2 = mpool.tile([128, FREE], f16, tag="w2")
                up = mpool.tile([128, FREE], f16, tag="up")
                vp = mpool.tile([128, FREE], f16, tag="vp")
                xx = tpool.tile([128, FREE], f16, tag="xx")
                yy = tpool.tile([128, FREE], f16, tag="yy")
                nc.vector.tensor_add(w1[:], x, y)
                nc.vector.tensor_sub(w2[:], x, y)
                nc.vector.tensor_mul(vp[:], x, y)
                nc.vector.tensor_mul(xx[:], x, x)
                nc.vector.tensor_mul(yy[:], y, y)
                nc.vector.tensor_add(up[:], xx[:], yy[:])
                maps = [w1, w2, up, vp]

                if stage < 1:
                    continue
                # ---- stage 1: column convs (transposed out), evict to fp16
                cm5s, cm11s = [], []
                for mp in range(4):
                    cm5 = cmpool.tile([128, FREE], f16, tag=f"cm5_{mp}")
                    cm11 = cmpool.tile([128, FREE], f16, tag=f"cm11_{mp}")
                    for u in range(NCH):
                        Kw, ws = KSZ[u], STARTS[u]
                        ps5 = ps1pool.tile([128, 512], f32, tag="ps5")
                        ps11 = ps1pool.tile([128, 512], f32, tag="ps11")
                        for c in range(NCH):
                            Kc, Mc = KSZ[c], MSZ[c]
                            lhs = maps[mp][0:Kc, 512 * c + ws : 512 * c + ws + Kw]
                            nc.tensor.matmul(
                                ps5[0:Kw, 118 * c : 118 * c + Mc],
                                lhs, w5[0:Kc, c, 0:Mc],
                                start=True, stop=True,
                            )
                            nc.tensor.matmul(
                                ps11[0:Kw, 118 * c : 118 * c + Mc],
                                lhs, w11[0:Kc, c, 0:Mc],
                                start=True, stop=True,
                            )
                        nc.scalar.copy(cm5[0:Kw, 512 * u : 512 * u + 512], ps5[0:Kw, :])
                        nc.scalar.copy(cm11[0:Kw, 512 * u : 512 * u + 512], ps11[0:Kw, :])
                    cm5s.append(cm5)
                    cm11s.append(cm11)

                # ---- stage 2 + window math
                if stage < 2:
                    continue
                for (srcs, wrow) in ((cm5s, w11), (cm11s, w5), (cm11s, w11)):
                    for u in range(NCH):
                        Kw, Mu = KSZ[u], MSZ[u]
                        pss = []
                        for mp in range(4):
                            ps = ps2pool.tile([128, 512], f32, tag="ps2")
                            nc.tensor.matmul(
                                ps[0:Mu, :],
                                wrow[0:Kw, u, 0:Mu],
                                srcs[mp][0:Kw, 512 * u : 512 * u + 512],
                                start=True, stop=True,
                            )
                            pss.append(ps)
                        S, D, Up, Vp = pss
                        if stage < 3:
                            continue
                        sa = wpool.tile([128, 512], f16, tag="sa")
                        sb = wpool.tile([128, 512], f16, tag="sb")
                        nc.scalar.activation(sa[0:Mu, :], S[0:Mu, :], Act.Square, scale=SQH)
                        nc.scalar.activation(sb[0:Mu, :], D[0:Mu, :], Act.Square, scale=SQH)
                        if stage < 4:
                            continue
                        tp = wpool.tile([128, 512], f16, tag="tp")
                        q = wpool.tile([128, 512], f16, tag="q")
                        nc.vector.tensor_sub(tp[0:Mu, :], sa[0:Mu, :], sb[0:Mu, :])
                        nc.vector.tensor_add(q[0:Mu, :], sa[0:Mu, :], sb[0:Mu, :])
                        f2p = wpool.tile([128, 512], f16, tag="f2p")
                        f2 = wpool.tile([128, 512], f16, tag="f2")
                        num = wpool.tile([128, 512], f16, tag="num")
                        d2 = wpool.tile([128, 512], f16, tag="d2")
                        den = wpool.tile([128, 512], f32, tag="den")
                        # f2 = (2V - T') + C2
                        nc.vector.scalar_tensor_tensor(
                            f2p[0:Mu, :], Vp[0:Mu, :], 2.0, tp[0:Mu, :],
                            op0=Alu.mult, op1=Alu.subtract)
                        nc.vector.tensor_scalar_add(f2[0:Mu, :], f2p[0:Mu, :], C2)
                        # num = (T' + C1) * f2
                        nc.vector.scalar_tensor_tensor(
                            num[0:Mu, :], tp[0:Mu, :], C1, f2[0:Mu, :],
                            op0=Alu.add, op1=Alu.mult)
                        # d2 = (U + C2) - q
                        nc.vector.scalar_tensor_tensor(
                            d2[0:Mu, :], Up[0:Mu, :], C2, q[0:Mu, :],
                            op0=Alu.add, op1=Alu.subtract)
                        # den = (q + C1) * d2   (fp32)
                        nc.vector.scalar_tensor_tensor(
                            den[0:Mu, :], q[0:Mu, :], C1, d2[0:Mu, :],
                            op0=Alu.add, op1=Alu.mult)
                        if stage < 5:
                            continue
                        r = wpool.tile([128, 512], f32, tag="r")
                        nc.vector.reciprocal(r[0:Mu, :], den[0:Mu, :])
                        s = wpool.tile([128, 512], f16, tag="s")
                        nc.vector.tensor_mul(s[0:Mu, :], num[0:Mu, :], r[0:Mu, :])
                        nc.vector.tensor_add(acc[0:Mu, :], acc[0:Mu, :], s[0:Mu, :])

            nc.sync.dma_start(out_d[:], acc[:])

    # Bacc defers register allocation / sync-wait legalization to compile();
    # run_bass_via_pjrt does not call finalize, so do it here.
    nc.finalize()
    return nc


def _get_program(reps=1, noop=False, stage=5):
    global _PROGRAM
    key = (reps, noop, stage)
    if not isinstance(_PROGRAM, dict):
        globals()['_PROGRAM'] = {}
    if key not in _PROGRAM:
        _PROGRAM[key] = _build_program(reps=reps, noop=noop, stage=stage)
    return _PROGRAM[key]


def _make_in_maps(img1, img2):
    x = np.asarray(img1)[:, :3].astype(np.float32)
    y = np.asarray(img2)[:, :3].astype(np.float32)
    wb = np.stack([_wblocks(5), _wblocks(11)], axis=1)  # [128, 2, 5, 118] fp16
    in_maps = []
    for i in range(8):
        xs = x[4 * i : 4 * i + 4].reshape(N_PLANES, 512, 512)
        ys = y[4 * i : 4 * i + 4].reshape(N_PLANES, 512, 512)
        xov = _overlap_planes(xs)
        yov = _overlap_planes(ys)
        xyov = np.concatenate([xov, yov], axis=2)  # [12, 128, 2*2560]
        in_maps.append({"xyov": xyov, "wb": wb})
    return in_maps


def _reduce_results(res):
    total = 0.0
    for i in range(8):
        total += np.asarray(res[i]["out"]).astype(np.float64).sum()
    npix = 32 * 3 * 512 * 512
    return np.float32(total / npix / 3.0)


def kernel(img1, img2):
    from concourse.bass_utils import run_bass_kernel_spmd

    in_maps = _make_in_maps(img1, img2)
    nc = _get_program()
    res = run_bass_kernel_spmd(nc, in_maps, core_ids=list(range(8))).results
    return _reduce_results(res)



# revision 8
# speedup vs baseline: 1.5851x; 1.5851x over previous
"""Distorted-SSIM loss kernel for Trainium2 (8 NeuronCores, data parallel).

v2 — engine-balanced rewrite of the v1 baseline (1.46 ms/core).

Decomposition per [512,512] plane (x, y = img planes):
    S = x+y, D = x-y, U = x^2+y^2, V2 = 2xy  (4 maps to blur)
    After separable blur (col conv then row conv, both as banded matmuls):
      sa = 0.5*Sb^2, sb = 0.5*Db^2
      a  = sa - sb + C1          (= 2 mu1 mu2 + C1)
      q  = sa + sb + C1          (= mu1^2 + mu2^2 + C1)
      num = a * (V2b + C12 - a);  den = q * (Ub + C12 - q);  C12 = C1+C2
      ssim = num/den; loss = mean over pixels, 3 window combos, planes.

Key changes vs v1:
  - reciprocal via reciprocal_approx_fast (1 custom DVE op) instead of the
    iterative DVE reciprocal (3.3us -> ~0.6us per tile).
  - multiply+accumulate via tensor_tensor_reduce: per-tile column sums land
    in a [128, 192] fp32 matrix, host reduces (no f16 accumulator).
  - elementwise work spread across DVE / Pool(gpsimd) / ScalarE:
      ScE: xx, yy, sa, sb (Square), 40 PSUM evictions (Copy)
      Pool (no PSUM, TT only): a0 = sa-sb, q0 = sa+sb
      DVE: w1, w2, vp2, up, e, e2, num, den, r, sred
"""

import sys
import numpy as np

for _p in ("/opt/trn_rl_repo",):
    if _p not in sys.path:
        sys.path.insert(0, _p)

SIGMA = 1.5
C1 = 0.01**2
C2 = 0.03**2
C12 = C1 + C2

STARTS = [0, 113, 231, 349, 467]
NCH = 5
KSZ = [min(128, 512 - s) for s in STARTS]
MSZ = [118, 118, 118, 118, 40]
N_PLANES = 12
FREE = NCH * 512  # 2560
NTILES = 15 * N_PLANES  # sred columns (3 combos x 5 u per plane)
OUTW = 192  # padded


def _gaussian(n, sigma=SIGMA):
    x = np.arange(n, dtype=np.float64)
    g = np.exp(-((x - n // 2) ** 2) / (2.0 * sigma**2))
    return (g / g.sum()).astype(np.float32)


def _norm_fp16_taps(g):
    """fp16 taps ULP-nudged so the fp64 sum is exactly 1.0."""
    t = g.astype(np.float16)
    for _ in range(500):
        td = t.astype(np.float64)
        err = td.sum() - 1.0
        if abs(err) < 2e-8:
            break
        bits = t.view(np.uint16).astype(np.int32) + (1 if err < 0 else -1)
        stepped = bits.astype(np.uint16).view(np.float16)
        delta = stepped.astype(np.float64) - td
        ad = np.abs(delta)
        ok = ad <= abs(err) * 1.000001
        i = int(np.argmax(np.where(ok, ad, -1.0))) if ok.any() else int(np.argmin(ad))
        t[i] = stepped[i]
    return t


def _wblocks(k):
    """Banded conv blocks [128, 5, 118]: W[kk, c, m] = g[in - out + pad]."""
    g = _norm_fp16_taps(_gaussian(k)).astype(np.float32)
    p = k // 2
    W = np.zeros((128, NCH, 118), np.float32)
    kk = np.arange(128)
    for c, s in enumerate(STARTS):
        m = np.arange(MSZ[c])
        j = (s + kk[:, None]) - (118 * c + m[None, :]) + p
        valid = (j >= 0) & (j < k) & (kk[:, None] < KSZ[c])
        W[:, c, : MSZ[c]][valid] = g[np.clip(j, 0, k - 1)][valid]
    return W


def _overlap_planes(pl):
    """[12, 512, 512] fp32 -> [12, 128, 5*512] fp16 overlapped h-window tiles."""
    t = np.zeros((N_PLANES, NCH, 128, 512), np.float32)
    for c, s in enumerate(STARTS):
        t[:, c, : KSZ[c], :] = pl[:, s : s + KSZ[c], :]
    return np.ascontiguousarray(
        t.transpose(0, 2, 1, 3).reshape(N_PLANES, 128, NCH * 512)
    ).astype(np.float16)


_PROGRAM = {}


def _build_program():
    import concourse.bass as bass
    import concourse.mybir as mybir
    from concourse import bacc, tile

    f32 = mybir.dt.float32
    f32r = mybir.dt.float32r
    f16 = mybir.dt.float16
    Alu = mybir.AluOpType
    Act = mybir.ActivationFunctionType

    nc = bacc.Bacc(None, target_bir_lowering=False)
    xy_d = nc.dram_tensor("xyov", [N_PLANES, 128, 2 * FREE], f16, kind="ExternalInput")
    wb_d = nc.dram_tensor("wb", [128, 2, NCH, 118], f16, kind="ExternalInput")
    out_d = nc.dram_tensor("out", [128, OUTW], f32, kind="ExternalOutput")

    SQH = float(np.sqrt(0.5))

    with tile.TileContext(nc) as tc:
        with (
            tc.tile_pool(name="const", bufs=1) as cpool,
            tc.tile_pool(name="xy", bufs=2) as xypool,
            tc.tile_pool(name="maps", bufs=2) as mpool,
            tc.tile_pool(name="cm", bufs=1) as cmpool,
            tc.tile_pool(name="win", bufs=3) as wpool,
            tc.tile_pool(name="ps1", bufs=2, space="PSUM") as ps1pool,
            tc.tile_pool(name="ps2", bufs=4, space="PSUM") as ps2pool,
        ):
            wb = cpool.tile([128, 2, NCH, 118], f16, tag="wb")
            nc.sync.dma_start(wb[:], wb_d[:])
            w5 = wb[:, 0]
            w11 = wb[:, 1]
            wr = [w5, w11]
            ocols = cpool.tile([128, OUTW], f32, tag="ocols")
            nc.vector.memset(ocols[:], 0.0)

            # dummy matmul: absorb wb DMA wait on PE once
            dummy = ps2pool.tile([128, 512], f32, tag="ps2")
            nc.tensor.matmul(
                dummy[0:118, 0:118], wb[0:128, 0, 0, 0:118], wb[0:128, 0, 0, 0:118],
                start=True, stop=True,
            )

            # fp32r views of the fp32 cm tiles (built per plane below)
            for p in range(N_PLANES):
                xy = xypool.tile([128, 2 * FREE], f16, tag="xy")
                nc.sync.dma_start(xy[:], xy_d[p])
                x = xy[:, 0:FREE]
                y = xy[:, FREE : 2 * FREE]

                w1 = mpool.tile([128, FREE], f16, tag="w1")
                w2 = mpool.tile([128, FREE], f16, tag="w2")
                vp2 = mpool.tile([128, FREE], f16, tag="vp2")
                xx = mpool.tile([128, FREE], f16, tag="xx")
                yy = mpool.tile([128, FREE], f16, tag="yy")
                up = mpool.tile([128, FREE], f16, tag="up")
                nc.vector.tensor_add(w1[:], x, y)
                nc.vector.tensor_sub(w2[:], x, y)
                nc.vector.scalar_tensor_tensor(
                    vp2[:], x, 2.0, y, op0=Alu.mult, op1=Alu.mult
                )
                nc.scalar.activation(xx[:], x, Act.Square)
                nc.scalar.activation(yy[:], y, Act.Square)
                nc.vector.tensor_add(up[:], xx[:], yy[:])
                maps = [w1, w2, up, vp2]  # S, D, U, V2

                # ---- stage 1: column convs -> PSUM -> DMA evict to fp32 cm
                cms = []  # [map][tap] -> f16 cm tile [128, FREE]
                for mp in range(4):
                    cm5 = cmpool.tile([128, FREE], f16, tag=f"cm5_{mp}")
                    cm11 = cmpool.tile([128, FREE], f16, tag=f"cm11_{mp}")
                    for u in range(NCH):
                        Kw, ws = KSZ[u], STARTS[u]
                        ps5 = ps1pool.tile([128, 512], f32, tag="ps5")
                        ps11 = ps1pool.tile([128, 512], f32, tag="ps11")
                        for c in range(NCH):
                            Kc, Mc = KSZ[c], MSZ[c]
                            lhs = maps[mp][0:Kc, 512 * c + ws : 512 * c + ws + Kw]
                            nc.tensor.matmul(
                                ps5[0:Kw, 118 * c : 118 * c + Mc],
                                lhs, w5[0:Kc, c, 0:Mc],
                                start=True, stop=True,
                            )
                            nc.tensor.matmul(
                                ps11[0:Kw, 118 * c : 118 * c + Mc],
                                lhs, w11[0:Kc, c, 0:Mc],
                                start=True, stop=True,
                            )
                        nc.scalar.copy(cm5[0:Kw, 512 * u : 512 * u + 512], ps5[0:Kw, :])
                        nc.scalar.copy(cm11[0:Kw, 512 * u : 512 * u + 512], ps11[0:Kw, :])
                    cms.append((cm5, cm11))

                # ---- stage 2 + window math
                # combos: (colblur from tap, rowblur tap index)
                for ci, (srctap, rowtap) in enumerate(((0, 1), (1, 0), (1, 1))):
                    for u in range(NCH):
                        Kw, Mu = KSZ[u], MSZ[u]
                        pss = []
                        for mp in range(4):
                            ps = ps2pool.tile([128, 512], f32, tag="ps2")
                            cmt = cms[mp][srctap]
                            nc.tensor.matmul(
                                ps[0:Mu, :],
                                wr[rowtap][0:Kw, u, 0:Mu],
                                cmt[0:Kw, 512 * u : 512 * u + 512],
                                start=True, stop=True,
                            )
                            pss.append(ps)
                        S, D, Up, Vp = pss

                        sa = wpool.tile([128, 512], f16, tag="sa")
                        sb = wpool.tile([128, 512], f16, tag="sb")
                        nc.scalar.activation(sa[0:Mu, :], S[0:Mu, :], Act.Square, scale=SQH)
                        nc.scalar.activation(sb[0:Mu, :], D[0:Mu, :], Act.Square, scale=SQH)

                        # a0 = sa - sb (= 2mu1mu2), q0 = sa + sb; +C1 folded
                        # into the downstream STTs (Pool: plain TT on SBUF only)
                        a0 = wpool.tile([128, 512], f16, tag="a0")
                        q0 = wpool.tile([128, 512], f16, tag="q0")
                        nc.gpsimd.tensor_sub(a0[0:Mu, :], sa[0:Mu, :], sb[0:Mu, :])
                        nc.gpsimd.tensor_add(q0[0:Mu, :], sa[0:Mu, :], sb[0:Mu, :])

                        # e = (V2b + C2) - a0  (= 2*sigma12 + C2)
                        # e2 = (Ub + C2) - q0  (= sigma1^2 + sigma2^2 + C2)
                        e = wpool.tile([128, 512], f16, tag="e")
                        e2 = wpool.tile([128, 512], f16, tag="e2")
                        nc.vector.scalar_tensor_tensor(
                            e[0:Mu, :], Vp[0:Mu, :], C2, a0[0:Mu, :],
                            op0=Alu.add, op1=Alu.subtract)
                        nc.vector.scalar_tensor_tensor(
                            e2[0:Mu, :], Up[0:Mu, :], C2, q0[0:Mu, :],
                            op0=Alu.add, op1=Alu.subtract)

                        num = wpool.tile([128, 512], f16, tag="num")
                        den = wpool.tile([128, 512], f32, tag="den")
                        nc.vector.scalar_tensor_tensor(
                            num[0:Mu, :], a0[0:Mu, :], C1, e[0:Mu, :],
                            op0=Alu.add, op1=Alu.mult)
                        nc.vector.scalar_tensor_tensor(
                            den[0:Mu, :], q0[0:Mu, :], C1, e2[0:Mu, :],
                            op0=Alu.add, op1=Alu.mult)

                        r = wpool.tile([128, 512], f32, tag="r")
                        nc.vector.reciprocal_approx_fast(r[0:Mu, :], den[0:Mu, :])

                        t = p * 15 + ci * 5 + u
                        scratch = wpool.tile([128, 512], f16, tag="scr")
                        nc.vector.scalar_tensor_tensor(
                            scratch[0:Mu, :], num[0:Mu, :], 1.0, r[0:Mu, :],
                            op0=Alu.mult, op1=Alu.mult,
                            accum_out=ocols[0:Mu, t : t + 1],
                        )

            nc.sync.dma_start(out_d[:], ocols[:])

    nc.finalize()
    return nc


def _get_program():
    global _PROGRAM
    if not isinstance(_PROGRAM, dict):
        globals()["_PROGRAM"] = {}
    if "v2" not in _PROGRAM:
        _PROGRAM["v2"] = _build_program()
    return _PROGRAM["v2"]


def _make_in_maps(img1, img2):
    x = np.asarray(img1)[:, :3].astype(np.float32)
    y = np.asarray(img2)[:, :3].astype(np.float32)
    wb = np.stack([_wblocks(5), _wblocks(11)], axis=1).astype(np.float16)
    in_maps = []
    for i in range(8):
        xs = x[4 * i : 4 * i + 4].reshape(N_PLANES, 512, 512)
        ys = y[4 * i : 4 * i + 4].reshape(N_PLANES, 512, 512)
        xov = _overlap_planes(xs)
        yov = _overlap_planes(ys)
        xyov = np.concatenate([xov, yov], axis=2)  # [12, 128, 2*2560]
        in_maps.append({"xyov": xyov, "wb": wb})
    return in_maps


def _reduce_results(res):
    total = 0.0
    for i in range(8):
        total += np.asarray(res[i]["out"]).astype(np.float64).sum()
    npix = 32 * 3 * 512 * 512
    return np.float32(total / npix / 3.0)


def kernel(img1, img2):
    from concourse.bass_utils import run_bass_kernel_spmd

    in_maps = _make_in_maps(img1, img2)
    nc = _get_program()
    res = run_bass_kernel_spmd(nc, in_maps, core_ids=list(range(8))).results
    return _reduce_results(res)
